# revision 33
# baseline (speedup 1.0000x reference)
"""MoE-LoRA layer (nn_MoELoRALayer) as a Bass/Tile kernel for 8 Trainium2 cores.

Computation (per token n):
    logits = x @ W_router.T                    # [N, 8]
    combine = renorm(top2(softmax(logits)))    # [N, 8]
    h       = x @ A_cat.T                      # [N, 128]   (8 experts x rank 16)
    hw      = h * combine_expanded             # [N, 128]
    out     = x @ W_base.T + b + 2.0 * hw @ B_cat.T

Sharding: data-parallel over tokens (1024 per core), all weights replicated.
Matmul operands bf16 (cast host-side), fp32 PSUM accumulation, bf16 output
(cast back to fp32 host-side; quantization noise ~0.2% rms, well under the
2e-2 gate).

Structure per core (v4 — startup-bandwidth-aware wave schedule):
  o0 is processed as two "waves" of 8 accumulators x 256 fp32 cols (two accs
  per bank -> 4 banks, memset + start=False groups); o1..o7 as the proven
  512-col half-sweeps (4 accs x full bank, start=True, no memsets).

  Wave 0 (o0 cols 0:256) runs k-interleaved with phase 1 (fused h+router
  matmuls) while the x stream arrives. The 256-col W slice halves the
  startup-critical W bytes: the PE consumes (xt .26 + W .066 + arhm .034) MB
  per 1.36us k-step = 265 GB/s, below the ~320 GB/s the three DMA queues
  deliver, so the PE never starves once started. The input stream is issued
  in strict k-need order at ~0.26MB granularity round-robined over the three
  queues (first items placed on the HW queues; the SWDGE queue ramps slow),
  with bias cols 0:512 riding mid-stream so wave-0 drains can fold bias in.

  Phase 1 packs h+logits token-major, 3 slots per PSUM bank (banks 0-2);
  routing (top-2 renormalized softmax) runs on DVE token-major. Wave 1
  (o0 cols 256:512) reuses wave 0's banks right after per-acc bias-fold
  drains; it runs acc-major with PE transposes (hw -> j-major hwt), wave-0
  LoRA matmuls (into scratch slots in banks 0-3) and their one-op merges
  sprinkled through its blocks, so routing/transposes/merge all hide under
  wave-1 matmuls. Wave 1's own LoRA lands as appended matmuls, then bias-fold
  drains.

  Sweeps s=2..15 ((o, half) for o=1..7) alternate bank groups 0-3 / 4-7,
  LoRA opener (start=True) + 32-k accumulation + bias-fold bf16 drain.
  W_base.T streams once through a double-buffered SBUF residency; o0's
  half-layout slab reuses wsb[0]'s bytes via a flat view. The last sweep is
  acc-major with per-acc drain+store on the HW DMA queues so the tail after
  the final matmul is <1us; the SWDGE (gpsimd) queue gets no stores in the
  last sweeps so its exit drain overlaps real work.

Host-side layout prep (part of sharding):
    xt    [8, 128, 4, 1024] = x_shard.T, K-chunk major (contraction on parts)
    wtoh0 [2, 128, 16, 512]: wtoh0[h,p,kp,kk*256+c] = W^T[(2kp+kk)*128+p,
                             h*256+c]   (o0's two 256-col halves)
    wto   [8, 128, 32, 512] = W_base.T packed per 512-wide output tile
    arhm  [128, 32, 136]    = per-K-tile [A^T | W_router^T] fused moving
    bft   [128, 4096]       = 2.0 * B.transpose(0,2,1).reshape(128, 4096)
    identb [128, 128]       = bf16 identity for PE transposes
"""

import numpy as np

import concourse.bacc as bacc
import concourse.bass as bass
import concourse.mybir as mybir
import concourse.tile as tile
from concourse.bass_utils import run_bass_kernel_spmd

N_CORES = 8
D_IN = 4096
D_OUT = 4096
N_EXP = 8
R = 16
J = N_EXP * R           # 128
SCALING = 2.0
TOK = 1024              # tokens per core
K_TILES = D_IN // 128   # 32
N_TILES = TOK // 128    # 8
O_TILES = D_OUT // 512  # 8
ARH = J + N_EXP         # 136 fused h+router columns
SLOT = 168              # column pitch of arh slots inside a PSUM bank

F32 = mybir.dt.float32
BF16 = mybir.dt.bfloat16

_CACHE = {}


def _build_program(finalize=True):
    key = ("nc", finalize)
    if key in _CACHE:
        return _CACHE[key]

    nc = bacc.Bacc(trn_type="TRN2")

    xt_d = nc.dram_tensor("xt", [8, 128, 4, TOK], BF16, kind="ExternalInput")
    wtoh0_d = nc.dram_tensor("wtoh0", [2, 128, 16, 512], BF16,
                             kind="ExternalInput")
    wto_d = nc.dram_tensor("wto", [O_TILES, 128, K_TILES, 512], BF16,
                           kind="ExternalInput")
    arhm_d = nc.dram_tensor("arhm", [128, K_TILES, ARH], BF16, kind="ExternalInput")
    bft_d = nc.dram_tensor("bft", [J, D_OUT], BF16, kind="ExternalInput")
    # bias pre-broadcast host-side: a partition_broadcast DMA rides the slow
    # SWDGE path and lands ~70us in; a plain 2MB HW-queue DMA does not.
    bias_d = nc.dram_tensor("bias2d", [128, D_OUT], F32, kind="ExternalInput")
    idb_d = nc.dram_tensor("identb", [128, 128], BF16, kind="ExternalInput")
    out_d = nc.dram_tensor("out", [TOK, D_OUT], BF16, kind="ExternalOutput")

    out_ap = out_d[:]
    mm = nc.tensor.matmul
    X = mybir.AxisListType.X
    OP = mybir.AluOpType

    with tile.TileContext(nc) as tc:
        with (
            tc.tile_pool(name="res", bufs=1) as res,
            tc.tile_pool(name="outp", bufs=8) as outp,
            tc.tile_pool(name="rsm", bufs=2) as rsm,
            tc.tile_pool(name="ps", bufs=1, space="PSUM") as ps,
        ):
            arhm_sb = res.tile([128, K_TILES, ARH], BF16)
            idb_sb = res.tile([128, 128], BF16)

            xtiles = [
                res.tile([128, 4, TOK], BF16, name=f"xt_{c}") for c in range(8)
            ]
            xts = []
            for c in range(8):
                for kk in range(4):
                    xts.append(xtiles[c][:, kk, :])

            wsb = [
                res.tile([128, K_TILES, 512], BF16, name=f"wsb{i}")
                for i in range(2)
            ]
            # o0's half-layout slab lives in wsb[0]'s bytes via a flat view:
            # half h at flat cols [h*8192, (h+1)*8192), laid [kp(16), 512]
            # with the 512 = (kk, 256c).
            wsb0f = wsb[0].rearrange("p a b -> p (a b)")
            bias_sb = res.tile([128, D_OUT], F32)
            bft_sb = res.tile([J, D_OUT], BF16)
            hwt_sb = res.tile([J, TOK], BF16)
            osb0 = [
                res.tile([128, 256], F32, name=f"osb0_{n}") for n in range(N_TILES)
            ]

            # 8 fixed PSUM bank tiles; regions managed manually, reuse chained
            # by Tile's region-level deps in emission order.
            B = [ps.tile([128, 512], F32, name=f"bank{i}") for i in range(8)]
            # PE-transpose scratch: bf16 view of bank 3's first 256 bytes
            pt = B[3][:, 0:64].bitcast(BF16)

            def arh_ap(n):
                i, s = divmod(n, 3)
                return B[i][:, s * SLOT:s * SLOT + ARH]

            def acc01_ap(n):
                # wave 0/1 accumulators: banks 4-7, two 256-col accs per bank
                return B[4 + n // 2][:, (n % 2) * 256:(n % 2) * 256 + 256]

            def w0slice(h, k):
                off = h * 8192 + (k // 2) * 512 + (k % 2) * 256
                return wsb0f[:, off:off + 256]

            def l_slot(n):
                # wave-0 LoRA scratch: B0c0,B0c1,B1c0,B1c1,B2c0,B2c1,B3c1,
                # then n7 reuses B0c0 (after merge 0 read it).
                lay = [(0, 0), (0, 1), (1, 0), (1, 1), (2, 0), (2, 1), (3, 1),
                       (0, 0)]
                b, c = lay[n]
                return B[b][:, c * 256:c * 256 + 256]

            queues3 = [nc.sync, nc.scalar, nc.gpsimd]
            outq = [nc.scalar, nc.sync, nc.gpsimd]

            def oh(i):
                return slice(i * 256, (i + 1) * 256)

            # ---- DMA: critical stream in strict k-need order, fine first
            # chunks, greedily assigned to the queue with the earliest
            # estimated finish (the SWDGE queue starts later and runs slower).
            crit = []   # (bytes, emit_fn)

            def xt_item(c, kx, t0=0, t1=TOK):
                crit.append((128 * (t1 - t0) * 2, lambda q: q.dma_start(
                    out=xtiles[c][:, kx:kx + 1, t0:t1],
                    in_=xt_d[c, :, kx:kx + 1, t0:t1],
                )))

            def arhm_item(k0, k1):
                crit.append((128 * (k1 - k0) * ARH * 2, lambda q: q.dma_start(
                    out=arhm_sb[:, k0:k1, :], in_=arhm_d[:, k0:k1, :],
                )))

            def w0_item(kp, nkp=1):
                crit.append((128 * 512 * nkp * 2, lambda q: q.dma_start(
                    out=wsb0f[:, kp * 512:(kp + nkp) * 512],
                    in_=wtoh0_d[0].rearrange("p a b -> p (a b)")
                    [:, kp * 512:(kp + nkp) * 512],
                )))

            for k in range(K_TILES):
                c, kk = divmod(k, 4)
                if k == 0:
                    xt_item(0, 0, 0, 512)
                    xt_item(0, 0, 512, TOK)
                    arhm_item(0, 2)
                else:
                    xt_item(c, kk)
                if k == 1:
                    arhm_item(2, 8)
                if k in (8, 16, 24):
                    arhm_item(k, k + 8)
                if k % 4 == 0:
                    w0_item(k // 2, 2)
            # greedy min-finish assignment; (start_us, GB_per_us)
            qstate = [[8.3, 0.107], [8.3, 0.107], [10.0, 0.075]]
            for nbytes, emit in crit:
                qi = min(range(3), key=lambda j: qstate[j][0])
                qstate[qi][0] += nbytes / 1e3 / qstate[qi][1] / 1e3
                emit(queues3[qi])
            # behind the stream: identity (first transpose ~67us), bias cols
            # 0:512 (first bias-fold drain ~62us)
            nc.gpsimd.dma_start(out=idb_sb, in_=idb_d[:])
            nc.scalar.dma_start(out=bias_sb[:, 0:512], in_=bias_d[:, 0:512])

            # Post-stream, deadline order: o0's second half (wave 1, ~57us),
            # bft (L-matmuls ~62us), bias cols 512: (sweep-2 drains ~130us).
            for cc in range(4):
                queues3[cc % 3].dma_start(
                    out=wsb0f[:, 8192 + cc * 2048:8192 + (cc + 1) * 2048],
                    in_=wtoh0_d[1].rearrange("p a b -> p (a b)")
                    [:, cc * 2048:(cc + 1) * 2048],
                )
            nc.sync.dma_start(out=bft_sb, in_=bft_d[:])
            nc.scalar.dma_start(
                out=bias_sb[:, 512:D_OUT], in_=bias_d[:, 512:D_OUT]
            )

            # ---- phase A: phase 1 (fused h+router) + wave 0, k-interleaved
            for i in range(3):
                nc.vector.memset(B[i], 0.0)
            for i in range(4, 8):
                nc.vector.memset(B[i], 0.0)



            for k in range(K_TILES):
                last_k = k == K_TILES - 1
                if last_k:
                    # close wave-0's accumulators first: n0's drain+memset on
                    # DVE then overlaps the remaining k=31 matmuls, so wave 1
                    # starts without waiting on the drain chain.
                    for n in range(N_TILES):
                        mm(acc01_ap(n), xts[k][:, n * 128:(n + 1) * 128],
                           w0slice(0, k), start=False, stop=True,
                           skip_group_check=True)
                for n in range(N_TILES):
                    mm(arh_ap(n), xts[k][:, n * 128:(n + 1) * 128],
                       arhm_sb[:, k, :], start=False, stop=last_k,
                       skip_group_check=True)
                if not last_k:
                    for n in range(N_TILES):
                        mm(acc01_ap(n), xts[k][:, n * 128:(n + 1) * 128],
                           w0slice(0, k), start=False, stop=False,
                           skip_group_check=True)

            # ---- routing: top-2 renormalized softmax, token-major per n ----
            hw_sbs = []

            def emit_routing_n(n):
                i, s = divmod(n, 3)
                lg = B[i][:, s * SLOT + J:s * SLOT + ARH]     # [128, 8] PSUM
                sc = rsm.tile([128, 8], F32, tag="rsc", bufs=4)
                rv = rsm.tile([128, 6, N_EXP], F32, tag="rv8", bufs=4)
                m1, m2, e2, den, rec = (sc[:, j:j + 1] for j in range(5))
                t, eq, msk, et, ge, w = (rv[:, j, :] for j in range(6))
                nc.vector.tensor_reduce(m1, lg, axis=X, op=OP.max)
                nc.vector.tensor_scalar(
                    out=t, in0=lg, scalar1=m1, scalar2=None,
                    op0=OP.subtract,
                )
                nc.vector.tensor_scalar(
                    out=eq, in0=t, scalar1=0.0, scalar2=None, op0=OP.is_ge
                )
                nc.vector.scalar_tensor_tensor(
                    out=msk, in0=eq, scalar=-1e30, in1=t,
                    op0=OP.mult, op1=OP.add,
                )
                nc.vector.tensor_reduce(m2, msk, axis=X, op=OP.max)
                nc.scalar.activation(e2, m2, mybir.ActivationFunctionType.Exp)
                nc.vector.tensor_scalar_add(den, e2, 1.0)
                nc.vector.reciprocal(rec, den)
                nc.scalar.activation(et, t, mybir.ActivationFunctionType.Exp)
                nc.vector.tensor_scalar(
                    out=ge, in0=t, scalar1=m2, scalar2=None, op0=OP.is_ge
                )
                nc.vector.tensor_tensor(out=w, in0=et, in1=ge, op=OP.mult)
                cmb = rsm.tile([128, N_EXP], F32, tag="cmb",
                               name=f"cmb_{n}", bufs=8)
                nc.vector.tensor_scalar_mul(cmb, w, rec)
                hw = rsm.tile([128, N_EXP, R], BF16, tag="hw",
                              name=f"hw_{n}", bufs=8)
                nc.vector.tensor_tensor(
                    out=hw,
                    in0=B[i][:, s * SLOT:s * SLOT + J].rearrange(
                        "p (e r) -> p e r", r=R
                    ),
                    in1=cmb.broadcast_to([128, N_EXP, R]),
                    op=OP.mult,
                )
                hw_sbs.append(hw)

            def emit_tpose(n):
                nc.tensor.transpose(
                    out=pt, in_=hw_sbs[n].rearrange("p e r -> p (e r)"),
                    identity=idb_sb,
                )
                nc.vector.tensor_copy(
                    out=hwt_sb[:, n * 128:(n + 1) * 128], in_=pt
                )

            def emit_lmerge(n):
                # wave-0 LoRA into scratch + one-op merge (bias already in
                # osb0 from the drain) -> bf16 -> store
                mm(l_slot(n), hwt_sb[:, n * 128:(n + 1) * 128],
                   bft_sb[:, oh(0)], start=False, stop=True,
                   skip_group_check=True)
                ob = outp.tile([128, 256], BF16, tag="obh", name=f"ob0_{n}")
                nc.vector.tensor_tensor(
                    out=ob, in0=osb0[n], in1=l_slot(n), op=OP.add
                )
                outq[n % 2].dma_start(
                    out=out_ap[n * 128:(n + 1) * 128, oh(0)], in_=ob
                )

            # ---- wave 1 (o0 cols 256:512), acc-major, banks 4-7 reused ----
            # wto[1] prefetch rides behind the o0 stream.
            for cc in range(2):
                queues3[cc % 3].dma_start(
                    out=wsb[1][:, cc * 16:(cc + 1) * 16, :],
                    in_=wto_d[1, :, cc * 16:(cc + 1) * 16, :],
                )
            # per-acc: fold-bias drain of wave 0, memset, routing (DVE);
            # k-loop (PE) with transposes/L-merges/L-memsets sprinkled.
            def pair(n):
                # fold-bias drain of wave-0 acc n + memset for wave-1's reuse
                nc.vector.tensor_tensor(
                    out=osb0[n], in0=acc01_ap(n), in1=bias_sb[:, oh(0)],
                    op=OP.add,
                )
                nc.vector.memset(acc01_ap(n), 0.0)

            # l_slot(i) overlaps phase-1 logits slots; memset only after the
            # routings that read them (keyed by block: emitted post-routing_n)
            MSL_AT = {1: [0], 2: [1], 4: [2], 5: [3], 7: [4, 5]}

            pair(0)
            for n in range(N_TILES):
                for k in range(K_TILES):
                    mm(acc01_ap(n), xts[k][:, n * 128:(n + 1) * 128],
                       w0slice(1, k), start=False, stop=False,
                       skip_group_check=True)
                if n + 1 < N_TILES:
                    pair(n + 1)   # block n+1 waits only its own pair
                emit_routing_n(n)
                for i in MSL_AT.get(n, []):
                    nc.vector.memset(l_slot(i), 0.0)
                if n >= 2:
                    emit_tpose(n - 2)
                if n >= 4:
                    emit_lmerge(n - 4)
            emit_tpose(6)
            emit_tpose(7)
            nc.vector.memset(l_slot(6), 0.0)
            for n in range(4, 7):
                emit_lmerge(n)
            nc.vector.memset(l_slot(7), 0.0)   # after merge 0 read B0c0
            emit_lmerge(7)

            # wave 1's LoRA appends close its groups, then bias-fold drains.
            for n in range(N_TILES):
                mm(acc01_ap(n), hwt_sb[:, n * 128:(n + 1) * 128],
                   bft_sb[:, oh(1)], start=False, stop=True,
                   skip_group_check=True)
            for n in range(N_TILES):
                ob = outp.tile([128, 256], BF16, tag="obh", name=f"ob1_{n}")
                nc.vector.tensor_tensor(
                    out=ob, in0=acc01_ap(n), in1=bias_sb[:, oh(1)], op=OP.add
                )
                outq[(1 + n) % 2].dma_start(
                    out=out_ap[n * 128:(n + 1) * 128, oh(1)], in_=ob
                )

            # ---- sweeps 2..15: (o, half) for o=1..7, 4 accs x 512 cols ----
            for s in range(2, 2 * O_TILES):
                o, half = divmod(s, 2)
                grp = B[0:4] if s % 2 == 0 else B[4:8]
                nset = [half * 4 + i for i in range(4)]
                osl = slice(o * 512, (o + 1) * 512)
                last = s == 2 * O_TILES - 1
                if half == 0 and o + 1 < O_TILES:
                    for cc in range(2):
                        queues3[(s + cc) % 3].dma_start(
                            out=wsb[(o + 1) % 2][:, cc * 16:(cc + 1) * 16, :],
                            in_=wto_d[o + 1, :, cc * 16:(cc + 1) * 16, :],
                        )
                if not last:
                    for i, n in enumerate(nset):
                        mm(grp[i], hwt_sb[:, n * 128:(n + 1) * 128],
                           bft_sb[:, osl], start=True, stop=False)
                    for k in range(K_TILES):
                        for i, n in enumerate(nset):
                            mm(grp[i], xts[k][:, n * 128:(n + 1) * 128],
                               wsb[o % 2][:, k, :], start=False,
                               stop=(k == K_TILES - 1))
                    for i, n in enumerate(nset):
                        ob = outp.tile([128, 512], BF16, tag="ob",
                                       name=f"ob{s}_{i}")
                        nc.vector.tensor_tensor(
                            out=ob, in0=grp[i], in1=bias_sb[:, osl], op=OP.add
                        )
                        # stores ride the two HW queues only; the SWDGE queue
                        # keeps input work so its exit drain stays short
                        outq[(s * 4 + i) % 2].dma_start(
                            out=out_ap[n * 128:(n + 1) * 128, osl], in_=ob
                        )
                else:
                    # last sweep acc-major: per-acc drain + store right after
                    # its k=31, so the post-matmul tail is one acc, not four.
                    for i, n in enumerate(nset):
                        mm(grp[i], hwt_sb[:, n * 128:(n + 1) * 128],
                           bft_sb[:, osl], start=True, stop=False)
                        for k in range(K_TILES):
                            mm(grp[i], xts[k][:, n * 128:(n + 1) * 128],
                               wsb[o % 2][:, k, :], start=False,
                               stop=(k == K_TILES - 1))
                        for hx in range(2):
                            ob = outp.tile([128, 256], BF16, tag="obh",
                                           name=f"obL_{i}_{hx}")
                            nc.vector.tensor_tensor(
                                out=ob, in0=grp[i][:, hx * 256:(hx + 1) * 256],
                                in1=bias_sb[:, o * 512 + hx * 256:
                                            o * 512 + (hx + 1) * 256],
                                op=OP.add,
                            )
                            outq[(i * 2 + hx) % 2].dma_start(
                                out=out_ap[n * 128:(n + 1) * 128,
                                           o * 512 + hx * 256:
                                           o * 512 + (hx + 1) * 256],
                                in_=ob,
                            )

    if finalize:
        nc.finalize()
    _CACHE[key] = nc
    return nc


def _prep_inputs(x, W_base, b_base, W_router, A, B):
    """Shard + lay out inputs for the 8 cores. Returns list of in_maps."""
    import ml_dtypes
    bf16 = ml_dtypes.bfloat16
    x = np.asarray(x)
    W_base = np.asarray(W_base)
    b_base = np.asarray(b_base)
    W_router = np.asarray(W_router)
    A = np.asarray(A)
    B = np.asarray(B)
    x_flat = np.ascontiguousarray(x, dtype=np.float32).reshape(-1, D_IN)

    wt = W_base.astype(np.float32, copy=False).T            # [d_in, d_out]
    # wto[o, p, k, c] = W^T[k*128 + p, o*512 + c]  (partition-first pack)
    wto = np.ascontiguousarray(
        wt.reshape(K_TILES, 128, O_TILES, 512)
        .transpose(2, 1, 0, 3)
        .astype(bf16)
    )
    # wtoh0[h, p, kp, kk*256 + c] = W^T[(2kp+kk)*128 + p, h*256 + c]
    wtoh0 = np.ascontiguousarray(
        wt[:, 0:512].reshape(16, 2, 128, 2, 256)
        .transpose(3, 2, 0, 1, 4)
        .reshape(2, 128, 16, 512)
        .astype(bf16)
    )
    acat = A.astype(np.float32, copy=False).reshape(J, D_IN)
    at = acat.T.reshape(K_TILES, 128, J).transpose(1, 0, 2)  # [p, k, j]
    wrt = (
        W_router.astype(np.float32, copy=False)
        .T.reshape(K_TILES, 128, N_EXP)
        .transpose(1, 0, 2)
    )
    arhm = np.ascontiguousarray(
        np.concatenate([at, wrt], axis=2).astype(bf16)
    )
    bft = np.ascontiguousarray(
        (SCALING * B.astype(np.float32, copy=False).transpose(0, 2, 1)
         .reshape(J, D_OUT)).astype(bf16)
    )
    bias2d = np.ascontiguousarray(
        np.broadcast_to(b_base.astype(np.float32, copy=False), (128, D_OUT))
    )
    identb = np.eye(128, dtype=np.float32).astype(bf16)

    in_maps = []
    for c in range(N_CORES):
        shard = x_flat[c * TOK:(c + 1) * TOK]               # [1024, 4096]
        # xt[chunk, p, kk, t] = x^T[(chunk*4 + kk)*128 + p, t]
        xt = np.ascontiguousarray(
            shard.T.astype(bf16)
            .reshape(8, 4, 128, TOK)
            .transpose(0, 2, 1, 3)
        )
        in_maps.append({
            "xt": xt, "wtoh0": wtoh0, "wto": wto, "arhm": arhm, "bft": bft,
            "bias2d": bias2d, "identb": identb,
        })
    return in_maps


def _run(in_maps, trace=False, **kw):
    nc = _build_program()
    return run_bass_kernel_spmd(
        nc, in_maps, core_ids=list(range(N_CORES)), trace=trace, **kw
    )


def kernel(x, W_base, b_base, W_router, A, B):
    orig_shape = np.asarray(x).shape
    in_maps = _prep_inputs(x, W_base, b_base, W_router, A, B)
    res = _run(in_maps)
    shards = [
        np.asarray(res.results[c]["out"], dtype=np.float32)
        for c in range(N_CORES)
    ]
    out = np.concatenate(shards, axis=0)
    return out.reshape(*orig_shape[:-1], D_OUT)


# revision 34
# speedup vs baseline: 1.0015x; 1.0015x over previous
"""MoE-LoRA layer (nn_MoELoRALayer) as a Bass/Tile kernel for 8 Trainium2 cores.

Computation (per token n):
    logits = x @ W_router.T                    # [N, 8]
    combine = renorm(top2(softmax(logits)))    # [N, 8]
    h       = x @ A_cat.T                      # [N, 128]   (8 experts x rank 16)
    hw      = h * combine_expanded             # [N, 128]
    out     = x @ W_base.T + b + 2.0 * hw @ B_cat.T

Sharding: data-parallel over tokens (1024 per core), all weights replicated.
Matmul operands bf16 (cast host-side), fp32 PSUM accumulation, bf16 output
(cast back to fp32 host-side; quantization noise ~0.2% rms, well under the
2e-2 gate).

Structure per core (v4 — startup-bandwidth-aware wave schedule):
  o0 is processed as two "waves" of 8 accumulators x 256 fp32 cols (two accs
  per bank -> 4 banks, memset + start=False groups); o1..o7 as the proven
  512-col half-sweeps (4 accs x full bank, start=True, no memsets).

  Wave 0 (o0 cols 0:256) runs k-interleaved with phase 1 (fused h+router
  matmuls) while the x stream arrives. The 256-col W slice halves the
  startup-critical W bytes: the PE consumes (xt .26 + W .066 + arhm .034) MB
  per 1.36us k-step = 265 GB/s, below the ~320 GB/s the three DMA queues
  deliver, so the PE never starves once started. The input stream is issued
  in strict k-need order at ~0.26MB granularity round-robined over the three
  queues (first items placed on the HW queues; the SWDGE queue ramps slow),
  with bias cols 0:512 riding mid-stream so wave-0 drains can fold bias in.

  Phase 1 packs h+logits token-major, 3 slots per PSUM bank (banks 0-2);
  routing (top-2 renormalized softmax) runs on DVE token-major. Wave 1
  (o0 cols 256:512) reuses wave 0's banks right after per-acc bias-fold
  drains; it runs acc-major with PE transposes (hw -> j-major hwt), wave-0
  LoRA matmuls (into scratch slots in banks 0-3) and their one-op merges
  sprinkled through its blocks, so routing/transposes/merge all hide under
  wave-1 matmuls. Wave 1's own LoRA lands as appended matmuls, then bias-fold
  drains.

  Sweeps s=2..15 ((o, half) for o=1..7) alternate bank groups 0-3 / 4-7,
  LoRA opener (start=True) + 32-k accumulation + bias-fold bf16 drain.
  W_base.T streams once through a double-buffered SBUF residency; o0's
  half-layout slab reuses wsb[0]'s bytes via a flat view. The last sweep is
  acc-major with per-acc drain+store on the HW DMA queues so the tail after
  the final matmul is <1us; the SWDGE (gpsimd) queue gets no stores in the
  last sweeps so its exit drain overlaps real work.

Host-side layout prep (part of sharding):
    xt    [8, 128, 4, 1024] = x_shard.T, K-chunk major (contraction on parts)
    wtoh0 [2, 128, 16, 512]: wtoh0[h,p,kp,kk*256+c] = W^T[(2kp+kk)*128+p,
                             h*256+c]   (o0's two 256-col halves)
    wto   [8, 128, 32, 512] = W_base.T packed per 512-wide output tile
    arhm  [128, 32, 136]    = per-K-tile [A^T | W_router^T] fused moving
    bft   [128, 4096]       = 2.0 * B.transpose(0,2,1).reshape(128, 4096)
    identb [128, 128]       = bf16 identity for PE transposes
"""

import numpy as np

import concourse.bacc as bacc
import concourse.bass as bass
import concourse.mybir as mybir
import concourse.tile as tile
from concourse.bass_utils import run_bass_kernel_spmd

N_CORES = 8
D_IN = 4096
D_OUT = 4096
N_EXP = 8
R = 16
J = N_EXP * R           # 128
SCALING = 2.0
TOK = 1024              # tokens per core
K_TILES = D_IN // 128   # 32
N_TILES = TOK // 128    # 8
O_TILES = D_OUT // 512  # 8
ARH = J + N_EXP         # 136 fused h+router columns
SLOT = 168              # column pitch of arh slots inside a PSUM bank

F32 = mybir.dt.float32
BF16 = mybir.dt.bfloat16

_CACHE = {}


def _build_program(finalize=True):
    key = ("nc", finalize)
    if key in _CACHE:
        return _CACHE[key]

    nc = bacc.Bacc(trn_type="TRN2")

    xt_d = nc.dram_tensor("xt", [8, 128, 4, TOK], BF16, kind="ExternalInput")
    wtoh0_d = nc.dram_tensor("wtoh0", [2, 128, 16, 512], BF16,
                             kind="ExternalInput")
    wto_d = nc.dram_tensor("wto", [O_TILES, 128, K_TILES, 512], BF16,
                           kind="ExternalInput")
    arhm_d = nc.dram_tensor("arhm", [128, K_TILES, ARH], BF16, kind="ExternalInput")
    bft_d = nc.dram_tensor("bft", [J, D_OUT], BF16, kind="ExternalInput")
    # bias pre-broadcast host-side: a partition_broadcast DMA rides the slow
    # SWDGE path and lands ~70us in; a plain 2MB HW-queue DMA does not.
    bias_d = nc.dram_tensor("bias2d", [128, D_OUT], F32, kind="ExternalInput")
    idb_d = nc.dram_tensor("identb", [128, 128], BF16, kind="ExternalInput")
    out_d = nc.dram_tensor("out", [TOK, D_OUT], BF16, kind="ExternalOutput")

    out_ap = out_d[:]
    mm = nc.tensor.matmul
    X = mybir.AxisListType.X
    OP = mybir.AluOpType

    with tile.TileContext(nc) as tc:
        with (
            tc.tile_pool(name="res", bufs=1) as res,
            tc.tile_pool(name="outp", bufs=8) as outp,
            tc.tile_pool(name="rsm", bufs=2) as rsm,
            tc.tile_pool(name="ps", bufs=1, space="PSUM") as ps,
        ):
            arhm_sb = res.tile([128, K_TILES, ARH], BF16)
            idb_sb = res.tile([128, 128], BF16)

            xtiles = [
                res.tile([128, 4, TOK], BF16, name=f"xt_{c}") for c in range(8)
            ]
            xts = []
            for c in range(8):
                for kk in range(4):
                    xts.append(xtiles[c][:, kk, :])

            wsb = [
                res.tile([128, K_TILES, 512], BF16, name=f"wsb{i}")
                for i in range(2)
            ]
            # o0's half-layout slab lives in wsb[0]'s bytes via a flat view:
            # half h at flat cols [h*8192, (h+1)*8192), laid [kp(16), 512]
            # with the 512 = (kk, 256c).
            wsb0f = wsb[0].rearrange("p a b -> p (a b)")
            bias_sb = res.tile([128, D_OUT], F32)
            bft_sb = res.tile([J, D_OUT], BF16)
            hwt_sb = res.tile([J, TOK], BF16)
            osb0 = [
                res.tile([128, 256], F32, name=f"osb0_{n}") for n in range(N_TILES)
            ]

            # 8 fixed PSUM bank tiles; regions managed manually, reuse chained
            # by Tile's region-level deps in emission order.
            B = [ps.tile([128, 512], F32, name=f"bank{i}") for i in range(8)]
            # PE-transpose scratch: bf16 view of bank 3's first 256 bytes
            pt = B[3][:, 0:64].bitcast(BF16)

            def arh_ap(n):
                i, s = divmod(n, 3)
                return B[i][:, s * SLOT:s * SLOT + ARH]

            def acc01_ap(n):
                # wave 0/1 accumulators: banks 4-7, two 256-col accs per bank
                return B[4 + n // 2][:, (n % 2) * 256:(n % 2) * 256 + 256]

            def w0slice(h, k):
                off = h * 8192 + (k // 2) * 512 + (k % 2) * 256
                return wsb0f[:, off:off + 256]

            def l_slot(n):
                # wave-0 LoRA scratch: B0c0,B0c1,B1c0,B1c1,B2c0,B2c1,B3c1,
                # then n7 reuses B0c0 (after merge 0 read it).
                lay = [(0, 0), (0, 1), (1, 0), (1, 1), (2, 0), (2, 1), (3, 1),
                       (0, 0)]
                b, c = lay[n]
                return B[b][:, c * 256:c * 256 + 256]

            queues3 = [nc.sync, nc.scalar, nc.gpsimd]
            outq = [nc.scalar, nc.sync, nc.gpsimd]

            def oh(i):
                return slice(i * 256, (i + 1) * 256)

            # ---- DMA: critical stream in strict k-need order, fine first
            # chunks, greedily assigned to the queue with the earliest
            # estimated finish (the SWDGE queue starts later and runs slower).
            crit = []   # (bytes, emit_fn)

            def xt_item(c, kx, t0=0, t1=TOK):
                crit.append((128 * (t1 - t0) * 2, lambda q: q.dma_start(
                    out=xtiles[c][:, kx:kx + 1, t0:t1],
                    in_=xt_d[c, :, kx:kx + 1, t0:t1],
                )))

            def arhm_item(k0, k1):
                crit.append((128 * (k1 - k0) * ARH * 2, lambda q: q.dma_start(
                    out=arhm_sb[:, k0:k1, :], in_=arhm_d[:, k0:k1, :],
                )))

            def w0_item(kp, nkp=1):
                crit.append((128 * 512 * nkp * 2, lambda q: q.dma_start(
                    out=wsb0f[:, kp * 512:(kp + nkp) * 512],
                    in_=wtoh0_d[0].rearrange("p a b -> p (a b)")
                    [:, kp * 512:(kp + nkp) * 512],
                )))

            for k in range(K_TILES):
                c, kk = divmod(k, 4)
                if k == 0:
                    xt_item(0, 0, 0, 512)
                    xt_item(0, 0, 512, TOK)
                    arhm_item(0, 2)
                else:
                    xt_item(c, kk)
                if k == 1:
                    arhm_item(2, 8)
                if k in (8, 16, 24):
                    arhm_item(k, k + 8)
                if k % 4 == 0:
                    w0_item(k // 2, 2)
            # greedy min-finish assignment; (start_us, GB_per_us)
            qstate = [[8.3, 0.107], [8.3, 0.107], [10.0, 0.075]]
            for nbytes, emit in crit:
                qi = min(range(3), key=lambda j: qstate[j][0])
                qstate[qi][0] += nbytes / 1e3 / qstate[qi][1] / 1e3
                emit(queues3[qi])
            # behind the stream: identity (first transpose ~67us), bias cols
            # 0:512 (first bias-fold drain ~62us)
            nc.gpsimd.dma_start(out=idb_sb, in_=idb_d[:])
            nc.scalar.dma_start(out=bias_sb[:, 0:512], in_=bias_d[:, 0:512])

            # Post-stream, deadline order: o0's second half (wave 1, ~57us),
            # bft (L-matmuls ~62us), bias cols 512: (sweep-2 drains ~130us).
            for cc in range(4):
                queues3[cc % 3].dma_start(
                    out=wsb0f[:, 8192 + cc * 2048:8192 + (cc + 1) * 2048],
                    in_=wtoh0_d[1].rearrange("p a b -> p (a b)")
                    [:, cc * 2048:(cc + 1) * 2048],
                )
            nc.sync.dma_start(out=bft_sb, in_=bft_d[:])
            nc.scalar.dma_start(
                out=bias_sb[:, 512:D_OUT], in_=bias_d[:, 512:D_OUT]
            )

            # ---- phase A: phase 1 (fused h+router) + wave 0, k-interleaved
            for i in range(3):
                nc.vector.memset(B[i], 0.0)
            for i in range(4, 8):
                nc.vector.memset(B[i], 0.0)



            for k in range(K_TILES):
                last_k = k == K_TILES - 1
                if last_k:
                    # close wave-0's accumulators first: n0's drain+memset on
                    # DVE then overlaps the remaining k=31 matmuls, so wave 1
                    # starts without waiting on the drain chain.
                    for n in range(N_TILES):
                        mm(acc01_ap(n), xts[k][:, n * 128:(n + 1) * 128],
                           w0slice(0, k), start=False, stop=True,
                           skip_group_check=True)
                for n in range(N_TILES):
                    mm(arh_ap(n), xts[k][:, n * 128:(n + 1) * 128],
                       arhm_sb[:, k, :], start=False, stop=last_k,
                       skip_group_check=True)
                if not last_k:
                    for n in range(N_TILES):
                        mm(acc01_ap(n), xts[k][:, n * 128:(n + 1) * 128],
                           w0slice(0, k), start=False, stop=False,
                           skip_group_check=True)

            # ---- routing: top-2 renormalized softmax, token-major per n ----
            hw_sbs = []

            def emit_routing_n(n):
                i, s = divmod(n, 3)
                lg = B[i][:, s * SLOT + J:s * SLOT + ARH]     # [128, 8] PSUM
                sc = rsm.tile([128, 8], F32, tag="rsc", bufs=4)
                rv = rsm.tile([128, 6, N_EXP], F32, tag="rv8", bufs=4)
                m1, m2, e2, den, rec = (sc[:, j:j + 1] for j in range(5))
                t, eq, msk, et, ge, w = (rv[:, j, :] for j in range(6))
                nc.vector.tensor_reduce(m1, lg, axis=X, op=OP.max)
                nc.vector.tensor_scalar(
                    out=t, in0=lg, scalar1=m1, scalar2=None,
                    op0=OP.subtract,
                )
                nc.vector.tensor_scalar(
                    out=eq, in0=t, scalar1=0.0, scalar2=None, op0=OP.is_ge
                )
                nc.vector.scalar_tensor_tensor(
                    out=msk, in0=eq, scalar=-1e30, in1=t,
                    op0=OP.mult, op1=OP.add,
                )
                nc.vector.tensor_reduce(m2, msk, axis=X, op=OP.max)
                nc.scalar.activation(e2, m2, mybir.ActivationFunctionType.Exp)
                nc.vector.tensor_scalar_add(den, e2, 1.0)
                nc.vector.reciprocal(rec, den)
                nc.scalar.activation(et, t, mybir.ActivationFunctionType.Exp)
                nc.vector.tensor_scalar(
                    out=ge, in0=t, scalar1=m2, scalar2=None, op0=OP.is_ge
                )
                nc.vector.tensor_tensor(out=w, in0=et, in1=ge, op=OP.mult)
                cmb = rsm.tile([128, N_EXP], F32, tag="cmb",
                               name=f"cmb_{n}", bufs=8)
                nc.vector.tensor_scalar_mul(cmb, w, rec)
                hw = rsm.tile([128, N_EXP, R], BF16, tag="hw",
                              name=f"hw_{n}", bufs=8)
                nc.vector.tensor_tensor(
                    out=hw,
                    in0=B[i][:, s * SLOT:s * SLOT + J].rearrange(
                        "p (e r) -> p e r", r=R
                    ),
                    in1=cmb.broadcast_to([128, N_EXP, R]),
                    op=OP.mult,
                )
                hw_sbs.append(hw)

            def emit_tpose(n):
                nc.tensor.transpose(
                    out=pt, in_=hw_sbs[n].rearrange("p e r -> p (e r)"),
                    identity=idb_sb,
                )
                # copy on the mostly-idle scalar engine: wave-1's DVE stream
                # (routing+drains+merges) is oversubscribed vs its PE work
                nc.scalar.activation(
                    hwt_sb[:, n * 128:(n + 1) * 128], pt,
                    mybir.ActivationFunctionType.Copy,
                )

            def emit_lmerge(n):
                # wave-0 LoRA into scratch + one-op merge (bias already in
                # osb0 from the drain) -> bf16 -> store
                mm(l_slot(n), hwt_sb[:, n * 128:(n + 1) * 128],
                   bft_sb[:, oh(0)], start=False, stop=True,
                   skip_group_check=True)
                ob = outp.tile([128, 256], BF16, tag="obh", name=f"ob0_{n}")
                nc.vector.tensor_tensor(
                    out=ob, in0=osb0[n], in1=l_slot(n), op=OP.add
                )
                outq[n % 2].dma_start(
                    out=out_ap[n * 128:(n + 1) * 128, oh(0)], in_=ob
                )

            # ---- wave 1 (o0 cols 256:512), acc-major, banks 4-7 reused ----
            # wto[1] prefetch rides behind the o0 stream.
            for cc in range(2):
                queues3[cc % 3].dma_start(
                    out=wsb[1][:, cc * 16:(cc + 1) * 16, :],
                    in_=wto_d[1, :, cc * 16:(cc + 1) * 16, :],
                )
            # per-acc: fold-bias drain of wave 0, memset, routing (DVE);
            # k-loop (PE) with transposes/L-merges/L-memsets sprinkled.
            def pair(n):
                # fold-bias drain of wave-0 acc n + memset for wave-1's reuse
                nc.vector.tensor_tensor(
                    out=osb0[n], in0=acc01_ap(n), in1=bias_sb[:, oh(0)],
                    op=OP.add,
                )
                nc.vector.memset(acc01_ap(n), 0.0)

            # l_slot(i) overlaps phase-1 logits slots; memset only after the
            # routings that read them (keyed by block: emitted post-routing_n)
            MSL_AT = {1: [0], 2: [1], 4: [2], 5: [3], 7: [4, 5]}

            pair(0)
            for n in range(N_TILES):
                for k in range(K_TILES):
                    mm(acc01_ap(n), xts[k][:, n * 128:(n + 1) * 128],
                       w0slice(1, k), start=False, stop=False,
                       skip_group_check=True)
                if n + 1 < N_TILES:
                    pair(n + 1)   # block n+1 waits only its own pair
                emit_routing_n(n)
                for i in MSL_AT.get(n, []):
                    nc.vector.memset(l_slot(i), 0.0)
                if n >= 2:
                    emit_tpose(n - 2)
                if n >= 4:
                    emit_lmerge(n - 4)
            emit_tpose(6)
            emit_tpose(7)
            nc.vector.memset(l_slot(6), 0.0)
            for n in range(4, 7):
                emit_lmerge(n)
            nc.vector.memset(l_slot(7), 0.0)   # after merge 0 read B0c0
            emit_lmerge(7)

            # wave 1's LoRA appends close its groups, then bias-fold drains.
            for n in range(N_TILES):
                mm(acc01_ap(n), hwt_sb[:, n * 128:(n + 1) * 128],
                   bft_sb[:, oh(1)], start=False, stop=True,
                   skip_group_check=True)
            for n in range(N_TILES):
                ob = outp.tile([128, 256], BF16, tag="obh", name=f"ob1_{n}")
                nc.vector.tensor_tensor(
                    out=ob, in0=acc01_ap(n), in1=bias_sb[:, oh(1)], op=OP.add
                )
                outq[(1 + n) % 2].dma_start(
                    out=out_ap[n * 128:(n + 1) * 128, oh(1)], in_=ob
                )

            # ---- sweeps 2..15: (o, half) for o=1..7, 4 accs x 512 cols ----
            for s in range(2, 2 * O_TILES):
                o, half = divmod(s, 2)
                grp = B[0:4] if s % 2 == 0 else B[4:8]
                nset = [half * 4 + i for i in range(4)]
                osl = slice(o * 512, (o + 1) * 512)
                last = s == 2 * O_TILES - 1
                if half == 0 and o + 1 < O_TILES:
                    for cc in range(2):
                        queues3[(s + cc) % 3].dma_start(
                            out=wsb[(o + 1) % 2][:, cc * 16:(cc + 1) * 16, :],
                            in_=wto_d[o + 1, :, cc * 16:(cc + 1) * 16, :],
                        )
                if not last:
                    for i, n in enumerate(nset):
                        mm(grp[i], hwt_sb[:, n * 128:(n + 1) * 128],
                           bft_sb[:, osl], start=True, stop=False)
                    for k in range(K_TILES):
                        for i, n in enumerate(nset):
                            mm(grp[i], xts[k][:, n * 128:(n + 1) * 128],
                               wsb[o % 2][:, k, :], start=False,
                               stop=(k == K_TILES - 1))
                    for i, n in enumerate(nset):
                        ob = outp.tile([128, 512], BF16, tag="ob",
                                       name=f"ob{s}_{i}")
                        nc.vector.tensor_tensor(
                            out=ob, in0=grp[i], in1=bias_sb[:, osl], op=OP.add
                        )
                        # stores ride the two HW queues only; the SWDGE queue
                        # keeps input work so its exit drain stays short
                        outq[(s * 4 + i) % 2].dma_start(
                            out=out_ap[n * 128:(n + 1) * 128, osl], in_=ob
                        )
                else:
                    # last sweep acc-major: per-acc drain + store right after
                    # its k=31, so the post-matmul tail is one acc, not four.
                    for i, n in enumerate(nset):
                        mm(grp[i], hwt_sb[:, n * 128:(n + 1) * 128],
                           bft_sb[:, osl], start=True, stop=False)
                        for k in range(K_TILES):
                            mm(grp[i], xts[k][:, n * 128:(n + 1) * 128],
                               wsb[o % 2][:, k, :], start=False,
                               stop=(k == K_TILES - 1))
                        for hx in range(2):
                            ob = outp.tile([128, 256], BF16, tag="obh",
                                           name=f"obL_{i}_{hx}")
                            nc.vector.tensor_tensor(
                                out=ob, in0=grp[i][:, hx * 256:(hx + 1) * 256],
                                in1=bias_sb[:, o * 512 + hx * 256:
                                            o * 512 + (hx + 1) * 256],
                                op=OP.add,
                            )
                            outq[(i * 2 + hx) % 2].dma_start(
                                out=out_ap[n * 128:(n + 1) * 128,
                                           o * 512 + hx * 256:
                                           o * 512 + (hx + 1) * 256],
                                in_=ob,
                            )

    if finalize:
        nc.finalize()
    _CACHE[key] = nc
    return nc


def _prep_inputs(x, W_base, b_base, W_router, A, B):
    """Shard + lay out inputs for the 8 cores. Returns list of in_maps."""
    import ml_dtypes
    bf16 = ml_dtypes.bfloat16
    x = np.asarray(x)
    W_base = np.asarray(W_base)
    b_base = np.asarray(b_base)
    W_router = np.asarray(W_router)
    A = np.asarray(A)
    B = np.asarray(B)
    x_flat = np.ascontiguousarray(x, dtype=np.float32).reshape(-1, D_IN)

    wt = W_base.astype(np.float32, copy=False).T            # [d_in, d_out]
    # wto[o, p, k, c] = W^T[k*128 + p, o*512 + c]  (partition-first pack)
    wto = np.ascontiguousarray(
        wt.reshape(K_TILES, 128, O_TILES, 512)
        .transpose(2, 1, 0, 3)
        .astype(bf16)
    )
    # wtoh0[h, p, kp, kk*256 + c] = W^T[(2kp+kk)*128 + p, h*256 + c]
    wtoh0 = np.ascontiguousarray(
        wt[:, 0:512].reshape(16, 2, 128, 2, 256)
        .transpose(3, 2, 0, 1, 4)
        .reshape(2, 128, 16, 512)
        .astype(bf16)
    )
    acat = A.astype(np.float32, copy=False).reshape(J, D_IN)
    at = acat.T.reshape(K_TILES, 128, J).transpose(1, 0, 2)  # [p, k, j]
    wrt = (
        W_router.astype(np.float32, copy=False)
        .T.reshape(K_TILES, 128, N_EXP)
        .transpose(1, 0, 2)
    )
    arhm = np.ascontiguousarray(
        np.concatenate([at, wrt], axis=2).astype(bf16)
    )
    bft = np.ascontiguousarray(
        (SCALING * B.astype(np.float32, copy=False).transpose(0, 2, 1)
         .reshape(J, D_OUT)).astype(bf16)
    )
    bias2d = np.ascontiguousarray(
        np.broadcast_to(b_base.astype(np.float32, copy=False), (128, D_OUT))
    )
    identb = np.eye(128, dtype=np.float32).astype(bf16)

    in_maps = []
    for c in range(N_CORES):
        shard = x_flat[c * TOK:(c + 1) * TOK]               # [1024, 4096]
        # xt[chunk, p, kk, t] = x^T[(chunk*4 + kk)*128 + p, t]
        xt = np.ascontiguousarray(
            shard.T.astype(bf16)
            .reshape(8, 4, 128, TOK)
            .transpose(0, 2, 1, 3)
        )
        in_maps.append({
            "xt": xt, "wtoh0": wtoh0, "wto": wto, "arhm": arhm, "bft": bft,
            "bias2d": bias2d, "identb": identb,
        })
    return in_maps


def _run(in_maps, trace=False, **kw):
    nc = _build_program()
    return run_bass_kernel_spmd(
        nc, in_maps, core_ids=list(range(N_CORES)), trace=trace, **kw
    )


def kernel(x, W_base, b_base, W_router, A, B):
    orig_shape = np.asarray(x).shape
    in_maps = _prep_inputs(x, W_base, b_base, W_router, A, B)
    res = _run(in_maps)
    shards = [
        np.asarray(res.results[c]["out"], dtype=np.float32)
        for c in range(N_CORES)
    ]
    out = np.concatenate(shards, axis=0)
    return out.reshape(*orig_shape[:-1], D_OUT)
